# revision 13
# baseline (speedup 1.0000x reference)
"""NormLinearAttention Trainium2 kernel (8 NeuronCores, SPMD).

Math (per batch b):
  q = relu(x @ Wq + bq); k = relu(x @ Wk + bk); v = x @ Wv + bv; u = x @ Wu + bu
  kv[h,d,e] = sum_n k[h,n,d] v[h,n,e];  kv = abs_clamp(kv, 0.01, 100)
  a = q @ kv  (per head);  z = LN(a) * ln_w + ln_b;  out = (u * z) @ Wo + bo

Sharding: rows (b, n) flattened to 32768 rows; core c owns rows
[c*4096, (c+1)*4096) — exactly half of batch c//2.  The kv reduction
couples the two halves of each batch: partial kv is AllReduce'd over
core pairs [[0,1],[2,3],[4,5],[6,7]].

On-chip layout: activations are kept feature-major (host pre-transposes
x per shard), so every projection/attention matmul has its contraction
dim on partitions with no on-chip transposes.  Phase A (k/v projections
+ partial kv) sweeps the 1024 output features in two 512-wide halves so
compute starts after ~3 MB of DMA instead of 12 MB; k/v biases are
applied on the (otherwise idle) DVE instead of PE seed matmuls.
q/u are NOT spilled to DRAM: phase C computes each 512-token window's
q/u projection into SBUF right before its attention.  Windows 0/1 read
the still-resident phase-A xT and are emitted right after phase A so
the PE stays busy while the kv AllReduce runs; later windows re-stream
their x columns from DRAM (6 MB) into a 3-deep window ring, and xT's
64 KB/partition is recycled for phase C tiles when its pool closes.
LN column-stats are matmuls against a ones vector (dedicated 2-bank
PSUM ring); attn^2 rides the scalar engine straight from PSUM.
Compute dtype bf16 (fp32 PSUM accumulation), host pre-casts inputs.
The `rep` build parameter replicates the whole pass inside one NEFF —
used only by test.py's timing methodology.
"""

import numpy as np
import ml_dtypes

import concourse.bass as bass
import concourse.mybir as mybir
import concourse.tile as tile
from concourse import bacc
from concourse.bass_utils import run_bass_kernel_spmd

B, N, D, H = 4, 8192, 1024, 16
HD = D // H          # 64
P = 128
DC = D // P          # 8 dim chunks
NCORES = 8
R_FULL = B * N // NCORES   # 4096 rows per core
WIN = 512
EPS = 1e-5
GROUPS = [[0, 1], [2, 3], [4, 5], [6, 7]]
PIPE = 1  # phase C: out-projection trails attention by PIPE windows

bf16 = mybir.dt.bfloat16
f32 = mybir.dt.float32
AF = mybir.ActivationFunctionType
ALU = mybir.AluOpType
NPBF16 = ml_dtypes.bfloat16


def build(R=R_FULL, rep=1, sim=False):
    RT = R // P          # rowtiles
    NW = R // WIN        # windows
    RPW = WIN // P       # rowtiles per window (4)
    ndev = 1 if sim else NCORES

    nc = bacc.Bacc("TRN2", target_bir_lowering=False, debug=False,
                   enable_asserts=False, num_devices=ndev)

    xt_ext = nc.dram_tensor("xt", [DC, P, R], bf16, kind="ExternalInput").ap()
    w_ext = {n: nc.dram_tensor(n, [D, D], bf16, kind="ExternalInput").ap()
             for n in ("wk", "wv", "wq", "wu", "wo")}
    bkb_ext = nc.dram_tensor("bk_b", [P, D], f32, kind="ExternalInput").ap()
    bvb_ext = nc.dram_tensor("bv_b", [P, D], f32, kind="ExternalInput").ap()
    bob_ext = nc.dram_tensor("bo_b", [P, D], f32, kind="ExternalInput").ap()
    bqf_ext = nc.dram_tensor("bq_fm", [P, DC], f32, kind="ExternalInput").ap()
    buf_ext = nc.dram_tensor("bu_fm", [P, DC], f32, kind="ExternalInput").ap()
    lnw_ext = nc.dram_tensor("lnw_fm", [P, DC], f32, kind="ExternalInput").ap()
    lnb_ext = nc.dram_tensor("lnb_fm", [P, DC], f32, kind="ExternalInput").ap()
    out_ext = nc.dram_tensor("out", [R, D], f32, kind="ExternalOutput").ap()

    with tile.TileContext(nc, num_cores=ndev) as tc:
        with (
            tc.tile_pool(name="const", bufs=1) as cp,
            tc.tile_pool(name="wpool", bufs=3) as wp,
            tc.tile_pool(name="wps", bufs=4, space="PSUM") as wps,
            tc.tile_pool(name="accps", bufs=1, space="PSUM") as accps,
            tc.tile_pool(name="dram", bufs=1, space="DRAM") as dram,
            tc.tile_pool(name="small", bufs=2) as sp,
        ):
            # ---- constants ----
            ones128 = cp.tile([P, 1], bf16, name="ones128")
            nc.vector.memset(ones128[:], 1.0)

            for _rep in range(rep):
              # kv bounce buffers (one pair-AllReduce per feature half:
              # half 0's collective runs during the half-1 sweep)
              kv_in = [dram.tile([P, WIN], f32, name=f"kv_in{h}")
                       for h in range(2)]
              kv_out = [dram.tile([P, WIN], f32, name=f"kv_out{h}")
                        for h in range(2)]
              kv_blk = sp.tile([P, DC * P], bf16, name="kv_blk", bufs=1)

              with tc.tile_pool(name="pcq", bufs=2) as pcq:

                  def qu_window(w, xsrc):
                      """q/u projection for one 512-token window.
                      xsrc(c) -> [P, WIN] x columns, feature chunk c."""
                      qT = pcq.tile([P, DC, WIN], bf16, name="qT",
                                    tag="qT", bufs=2)
                      uT = pcq.tile([P, DC, WIN], bf16, name="uT",
                                    tag="uT", bufs=2)
                      # all of q first: wu's DMA may still be in flight
                      for wname, bias, func, dst in (
                          ("wq", bq_fm, AF.Relu, qT),
                          ("wu", bu_fm, AF.Identity, uT),
                      ):
                          for t in range(DC):
                              ps = wps.tile([P, WIN], f32, name="pqu",
                                            tag="work")
                              for c in range(DC):
                                  nc.tensor.matmul(
                                      ps[:],
                                      w_sb[wname][:, c, t * P:(t + 1) * P],
                                      xsrc(c),
                                      start=(c == 0), stop=(c == DC - 1))
                              nc.scalar.activation(dst[:, t, :], ps[:], func,
                                                   bias=bias[:, t:t + 1],
                                                   scale=1.0)
                      return qT, uT

                  with (
                      tc.tile_pool(name="xtp", bufs=1) as xtp,
                      tc.tile_pool(name="ab", bufs=2) as ab,
                  ):
                      # first half of wk/wv unblocks the first rowtile sweep
                      # critical path to the first rowtile: 2 weight-half
                      # descriptors + window 0's 8 xT pieces (descriptor
                      # issue is serial, so keep this prefix short)
                      w_sb = {}
                      for n in ("wk", "wv"):
                          t = wp.tile([P, DC, D], bf16, name=f"{n}_sb",
                                      tag="W")
                          nc.sync.dma_start(
                              t[:, :, 0:WIN],
                              w_ext[n].rearrange("(c p) n -> p c n",
                                                 p=P)[:, :, 0:WIN])
                          w_sb[n] = t

                      xT = [xtp.tile([P, R], bf16, name=f"xT{c}", tag=f"xT{c}")
                            for c in range(DC)]
                      for c in range(DC):
                          nc.sync.dma_start(xT[c][:, 0:WIN],
                                            xt_ext[c][:, 0:WIN])

                      # k/v bias rows broadcast to all partitions
                      bk_b = cp.tile([P, D], f32, name="bk_b")
                      nc.sync.dma_start(bk_b[:], bkb_ext)
                      bv_b = cp.tile([P, D], f32, name="bv_b")
                      nc.sync.dma_start(bv_b[:], bvb_ext)

                      # rest of xT in consumption (column-window) order
                      for w in range(1, NW):
                          for c in range(DC):
                              nc.sync.dma_start(
                                  xT[c][:, w * WIN:(w + 1) * WIN],
                                  xt_ext[c][:, w * WIN:(w + 1) * WIN])

                      # second half of wk/wv, then wq + phase-C params
                      for n in ("wk", "wv"):
                          nc.sync.dma_start(
                              w_sb[n][:, :, WIN:D],
                              w_ext[n].rearrange("(c p) n -> p c n",
                                                 p=P)[:, :, WIN:D])

                      t = wp.tile([P, DC, D], bf16, name="wq_sb", tag="W")
                      nc.sync.dma_start(
                          t[:], w_ext["wq"].rearrange("(c p) n -> p c n", p=P))
                      w_sb["wq"] = t

                      bo_b = cp.tile([P, D], f32, name="bo_b")
                      nc.sync.dma_start(bo_b[:], bob_ext)
                      bq_fm = cp.tile([P, DC], f32, name="bq_fm")
                      nc.sync.dma_start(bq_fm[:], bqf_ext)
                      bu_fm = cp.tile([P, DC], f32, name="bu_fm")
                      nc.sync.dma_start(bu_fm[:], buf_ext)
                      lnw_fm = cp.tile([P, DC], f32, name="lnw_fm")
                      nc.sync.dma_start(lnw_fm[:], lnw_ext)
                      lnb_fm = cp.tile([P, DC], f32, name="lnb_fm")
                      nc.sync.dma_start(lnb_fm[:], lnb_ext)

                      # ---- phase A: k, v projections + partial kv ----
                      # Two 512-wide feature halves; kv head-pair block g
                      # lives entirely in half g//4, so each half's kv
                      # accumulation is a self-contained PSUM bank group.
                      kv_ps = accps.tile([P, DC * P], f32, name="kv_ps")
                      for half in range(2):
                          hs = slice(half * WIN, half * WIN + WIN)
                          for rt in range(RT):
                              pk = wps.tile([P, WIN], f32, name="pk",
                                            tag="work")
                              pv = wps.tile([P, WIN], f32, name="pv",
                                            tag="work")
                              for c in range(DC):
                                  st, sto = c == 0, c == DC - 1
                                  lhs = xT[c][:, rt * P:(rt + 1) * P]
                                  nc.tensor.matmul(pk[:], lhs,
                                                   w_sb["wk"][:, c, hs],
                                                   start=st, stop=sto)
                                  nc.tensor.matmul(pv[:], lhs,
                                                   w_sb["wv"][:, c, hs],
                                                   start=st, stop=sto)
                              k_bf = ab.tile([P, WIN], bf16, name="k_bf",
                                             tag="kvt", bufs=4)
                              v_bf = ab.tile([P, WIN], bf16, name="v_bf",
                                             tag="kvt", bufs=4)
                              kf = ab.tile([P, WIN], f32, name="kf", tag="kf",
                                           bufs=3)
                              # k = relu(psum + bias): DVE add, then max0+cast
                              nc.vector.tensor_tensor(kf[:], pk[:],
                                                      bk_b[:, hs], ALU.add)
                              nc.vector.tensor_scalar(k_bf[:], kf[:], 0.0,
                                                      None, op0=ALU.max)
                              nc.vector.tensor_tensor(v_bf[:], pv[:],
                                                      bv_b[:, hs], ALU.add)
                              # partial kv per head-pair: [128,128] blocks
                              for g4 in range(4):
                                  g = half * 4 + g4
                                  nc.tensor.matmul(
                                      kv_ps[:, g * P:(g + 1) * P],
                                      k_bf[:, g4 * P:(g4 + 1) * P],
                                      v_bf[:, g4 * P:(g4 + 1) * P],
                                      start=(rt == 0 and g4 == 0),
                                      stop=(rt == RT - 1 and g4 == 3),
                                  )

                          # this half's kv partial -> DRAM -> pair AllReduce
                          kv_sb = sp.tile([P, WIN], f32, name="kv_sb", bufs=2)
                          nc.vector.tensor_copy(kv_sb[:], kv_ps[:, hs])
                          nc.sync.dma_start(kv_in[half][:], kv_sb[:])
                          if sim:
                              nc.sync.dma_start(kv_out[half][:],
                                                kv_in[half][:])
                          else:
                              nc.gpsimd.collective_compute(
                                  "AllReduce", ALU.add, replica_groups=GROUPS,
                                  ins=[kv_in[half][:]], outs=[kv_out[half][:]],
                              )

                      # wu/wo loads resolve their buffer-slot waits as phase A
                      # drains; emitted after the A loop so they can't block
                      # earlier DMA queue entries.
                      for n in ("wu", "wo"):
                          t = wp.tile([P, DC, D], bf16, name=f"{n}_sb",
                                      tag="W")
                          nc.sync.dma_start(
                              t[:],
                              w_ext[n].rearrange("(c p) n -> p c n", p=P))
                          w_sb[n] = t

                      # windows 0/1's q/u run during the collective, reading
                      # the still-resident phase-A xT
                      qu_tiles = {}
                      for w in range(min(2, NW)):
                          cols = slice(w * WIN, (w + 1) * WIN)
                          qu_tiles[w] = qu_window(
                              w, lambda c, cols=cols: xT[c][:, cols])

                  # ---- phase C (xT/ab space recycled from here on) ----
                  pc_cm = tc.tile_pool(name="pc", bufs=2)
                  pc = pc_cm.__enter__()
                  # x column-window ring for windows >= 2, re-streamed from
                  # DRAM; prefetched ahead of the collective-gated readback
                  # so these DMAs never queue behind it.
                  xw_tiles = {}

                  def xw_fetch(w):
                      if not 2 <= w < NW:
                          return
                      xw = pc.tile([P, DC, WIN], bf16, name="xw", tag="xw",
                                   bufs=2)
                      for c in range(DC):
                          nc.sync.dma_start(
                              xw[:, c, :],
                              xt_ext[c][:, w * WIN:(w + 1) * WIN])
                      xw_tiles[w] = xw

                  xw_fetch(2)
                  xw_fetch(3)

                  # kv: per-half readback, clamp, block-diagonal build
                  nc.vector.memset(kv_blk[:], 0.0)
                  for half in range(2):
                      kv_rb = sp.tile([P, WIN], f32, name="kv_rb", bufs=2)
                      nc.sync.dma_start(kv_rb[:], kv_out[half][:])
                      # clamp to [-100, 100], then |.| >= 0.01 keeping sign
                      nc.vector.tensor_scalar(kv_rb[:], kv_rb[:], -100.0,
                                              100.0, op0=ALU.max, op1=ALU.min)
                      kv_sgn = sp.tile([P, WIN], bf16, name="kv_sgn", bufs=2)
                      nc.scalar.activation(kv_sgn[:], kv_rb[:], AF.Sign)
                      nc.scalar.activation(kv_rb[:], kv_rb[:], AF.Abs)
                      nc.vector.tensor_scalar(kv_rb[:], kv_rb[:], 0.01, None,
                                              op0=ALU.max)
                      kv_cl = sp.tile([P, WIN], bf16, name="kv_cl", bufs=2)
                      nc.vector.tensor_tensor(kv_cl[:], kv_sgn[:], kv_rb[:],
                                              ALU.mult)
                      # kv_blk[0:64, g*128:g*128+64]       = kv(head 2g)
                      # kv_blk[64:128, g*128+64:g*128+128] = kv(head 2g+1)
                      for g4 in range(4):
                          g = half * 4 + g4
                          nc.vector.tensor_copy(
                              kv_blk[0:HD, g * P:g * P + HD],
                              kv_cl[0:HD, g4 * P:g4 * P + HD])
                          nc.vector.tensor_copy(
                              kv_blk[HD:P, g * P + HD:(g + 1) * P],
                              kv_cl[HD:P, g4 * P + HD:(g4 + 1) * P])

                  zw_tiles = {}
                  for w in range(NW + PIPE):
                    if w < NW:
                      qT_w, uT_w = qu_tiles.pop(w)

                      # attention + attn^2 (ACT, straight from PSUM)
                      attn = pc.tile([P, DC, WIN], bf16, name="attn",
                                     tag="attn")
                      at2 = pc.tile([P, DC, WIN], bf16, name="at2",
                                    tag="at2", bufs=1)
                      for g in range(DC):
                          aps = wps.tile([P, WIN], f32, name="aps", tag="work")
                          nc.tensor.matmul(aps[:],
                                           kv_blk[:, g * P:(g + 1) * P],
                                           qT_w[:, g, :], start=True,
                                           stop=True)
                          nc.scalar.activation(attn[:, g, :], aps[:], AF.Copy)
                          nc.vector.tensor_tensor(at2[:, g, :], attn[:, g, :],
                                                  attn[:, g, :], ALU.mult)
                      # LN stats: per-column sums over all 1024 dims
                      s_ps = wps.tile([1, WIN], f32, name="s_ps", tag="stat",
                                      bufs=2)
                      q_ps = wps.tile([1, WIN], f32, name="q_ps", tag="stat",
                                      bufs=2)
                      for g in range(DC):
                          nc.tensor.matmul(s_ps[:], ones128[:], attn[:, g, :],
                                           start=(g == 0), stop=(g == DC - 1))
                          nc.tensor.matmul(q_ps[:], ones128[:], at2[:, g, :],
                                           start=(g == 0), stop=(g == DC - 1))

                      # next window's q/u + x prefetch (emitted between the
                      # stats matmuls and the out-projection so the PE always
                      # has independent work while the LN chain runs)
                      if w + 2 < NW:
                          xw = xw_tiles.pop(w + 2)
                          qu_tiles[w + 2] = qu_window(
                              w + 2, lambda c, xw=xw: xw[:, c, :])
                      xw_fetch(w + 4)

                      mean_t = pc.tile([1, WIN], f32, name="mean_t",
                                       tag="mean_t")
                      var_t = pc.tile([1, WIN], f32, name="var_t", tag="var_t")
                      nc.vector.tensor_scalar(mean_t[:], s_ps[:], 1.0 / D,
                                              None, op0=ALU.mult)   # mean
                      # var + eps = E[x^2] - mean^2 + eps
                      nc.vector.tensor_tensor(var_t[:], mean_t[:], mean_t[:],
                                              ALU.mult)             # mean^2
                      nc.vector.scalar_tensor_tensor(var_t[:], q_ps[:],
                                                     1.0 / D, var_t[:],
                                                     ALU.mult, ALU.subtract)
                      nc.vector.tensor_scalar(var_t[:], var_t[:], EPS, None,
                                              op0=ALU.add)
                      nc.vector.reciprocal(var_t[:], var_t[:])
                      rstd = pc.tile([1, WIN], bf16, name="rstd", tag="rstd")
                      nc.scalar.activation(rstd[:], var_t[:], AF.Sqrt)
                      shp = pc.tile([1, WIN], bf16, name="shp", tag="shp")
                      # shiftpre = -mean * rstd
                      nc.vector.scalar_tensor_tensor(shp[:], mean_t[:], -1.0,
                                                     rstd[:], ALU.mult,
                                                     ALU.mult)
                      # broadcast per-column stats to all partitions (GPSIMD)
                      rstd_b = pc.tile([P, WIN], bf16, name="rstd_b",
                                       tag="rstd_b")
                      nc.gpsimd.partition_broadcast(rstd_b[:], rstd[:])
                      shp_b = pc.tile([P, WIN], bf16, name="shp_b",
                                      tag="shp_b")
                      nc.gpsimd.partition_broadcast(shp_b[:], shp[:])

                      # z = ((attn * rstd + shiftpre) * lnw + lnb) * u
                      zw = pc.tile([P, DC, WIN], bf16, name="zw", tag="zw",
                                   bufs=PIPE + 1)
                      for g in range(DC):
                          zt = pc.tile([P, WIN], bf16, name="zt", tag="zt",
                                       bufs=3)
                          nc.vector.tensor_tensor(zt[:], attn[:, g, :],
                                                  rstd_b[:], ALU.mult)
                          nc.vector.tensor_tensor(zt[:], zt[:], shp_b[:],
                                                  ALU.add)
                          nc.vector.tensor_scalar(zt[:], zt[:],
                                                  lnw_fm[:, g:g + 1],
                                                  lnb_fm[:, g:g + 1],
                                                  op0=ALU.mult, op1=ALU.add)
                          nc.vector.tensor_tensor(zw[:, g, :], zt[:],
                                                  uT_w[:, g, :], ALU.mult)
                      zw_tiles[w] = zw

                    if w >= PIPE:
                      # out = z @ Wo + bo for window w-PIPE (row-major out,
                      # zT chunks stationary)
                      wc = w - PIPE
                      zw = zw_tiles.pop(wc)
                      for j in range(RPW):
                          o0 = wps.tile([P, WIN], f32, name="o0", tag="work")
                          o1 = wps.tile([P, WIN], f32, name="o1", tag="work")
                          for c in range(DC):
                              lhs = zw[:, c, j * P:(j + 1) * P]
                              nc.tensor.matmul(o0[:], lhs,
                                               w_sb["wo"][:, c, 0:WIN],
                                               start=(c == 0),
                                               stop=(c == DC - 1))
                              nc.tensor.matmul(o1[:], lhs,
                                               w_sb["wo"][:, c, WIN:D],
                                               start=(c == 0),
                                               stop=(c == DC - 1))
                          osb = pc.tile([P, D], f32, name="osb", tag="osb",
                                        bufs=2)
                          nc.vector.scalar_tensor_tensor(osb[:, 0:WIN], o0[:],
                                                         1.0, bo_b[:, 0:WIN],
                                                         ALU.mult, ALU.add)
                          nc.vector.scalar_tensor_tensor(osb[:, WIN:D], o1[:],
                                                         1.0, bo_b[:, WIN:D],
                                                         ALU.mult, ALU.add)
                          rt = wc * RPW + j
                          nc.sync.dma_start(out_ext[rt * P:(rt + 1) * P, :],
                                            osb[:])

                  pc_cm.__exit__(None, None, None)

    nc.compile()
    return nc


def make_in_maps(query, Wq, bq, Wk, bk, Wv, bv, Wu, bu, Wo, bo, ln_w, ln_b,
                 R=R_FULL):
    xs = query.reshape(-1, D).astype(NPBF16)
    common = {
        "wk": np.ascontiguousarray(Wk).astype(NPBF16),
        "wv": np.ascontiguousarray(Wv).astype(NPBF16),
        "wq": np.ascontiguousarray(Wq).astype(NPBF16),
        "wu": np.ascontiguousarray(Wu).astype(NPBF16),
        "wo": np.ascontiguousarray(Wo).astype(NPBF16),
        "bk_b": np.ascontiguousarray(
            np.broadcast_to(bk.astype(np.float32), (P, D))),
        "bv_b": np.ascontiguousarray(
            np.broadcast_to(bv.astype(np.float32), (P, D))),
        "bo_b": np.ascontiguousarray(
            np.broadcast_to(bo.astype(np.float32), (P, D))),
        "bq_fm": np.ascontiguousarray(bq.astype(np.float32).reshape(DC, P).T),
        "bu_fm": np.ascontiguousarray(bu.astype(np.float32).reshape(DC, P).T),
        "lnw_fm": np.ascontiguousarray(ln_w.astype(np.float32).reshape(DC, P).T),
        "lnb_fm": np.ascontiguousarray(ln_b.astype(np.float32).reshape(DC, P).T),
    }
    return [dict(common, xt=np.ascontiguousarray(
                xs[c * R:(c + 1) * R].T.reshape(DC, P, R)))
            for c in range(NCORES)]


_NC_CACHE = {}


def kernel(query, Wq, bq, Wk, bk, Wv, bv, Wu, bu, Wo, bo, ln_w, ln_b):
    query = np.asarray(query, dtype=np.float32)
    if "nc" not in _NC_CACHE:
        _NC_CACHE["nc"] = build()
    nc = _NC_CACHE["nc"]
    in_maps = make_in_maps(query, np.asarray(Wq), np.asarray(bq),
                           np.asarray(Wk), np.asarray(bk),
                           np.asarray(Wv), np.asarray(bv),
                           np.asarray(Wu), np.asarray(bu),
                           np.asarray(Wo), np.asarray(bo),
                           np.asarray(ln_w), np.asarray(ln_b))
    res = run_bass_kernel_spmd(nc, in_maps, list(range(NCORES)))
    out = np.empty((B * N, D), np.float32)
    for c in range(NCORES):
        out[c * R_FULL:(c + 1) * R_FULL] = res.results[c]["out"]
    return out.reshape(B, N, D)


# revision 16
# speedup vs baseline: 1.0200x; 1.0200x over previous
"""NormLinearAttention Trainium2 kernel (8 NeuronCores, SPMD).

Math (per batch b):
  q = relu(x @ Wq + bq); k = relu(x @ Wk + bk); v = x @ Wv + bv; u = x @ Wu + bu
  kv[h,d,e] = sum_n k[h,n,d] v[h,n,e];  kv = abs_clamp(kv, 0.01, 100)
  a = q @ kv  (per head);  z = LN(a) * ln_w + ln_b;  out = (u * z) @ Wo + bo

Sharding: rows (b, n) flattened to 32768 rows; core c owns rows
[c*4096, (c+1)*4096) — exactly half of batch c//2.  The kv reduction
couples the two halves of each batch: partial kv is AllReduce'd over
core pairs [[0,1],[2,3],[4,5],[6,7]].

On-chip layout: activations are kept feature-major (host pre-transposes
x per shard), so every projection/attention matmul has its contraction
dim on partitions with no on-chip transposes.  Phase A (k/v projections
+ partial kv) sweeps the 1024 output features in two 512-wide halves so
compute starts after ~3 MB of DMA instead of 12 MB; k/v biases are
applied on the (otherwise idle) DVE instead of PE seed matmuls.
q/u are NOT spilled to DRAM: phase C computes each 512-token window's
q/u projection into SBUF right before its attention.  Windows 0/1 read
the still-resident phase-A xT and are emitted right after phase A so
the PE stays busy while the kv AllReduce runs; later windows re-stream
their x columns from DRAM (6 MB) into a 3-deep window ring, and xT's
64 KB/partition is recycled for phase C tiles when its pool closes.
LN column-stats are matmuls against a ones vector (dedicated 2-bank
PSUM ring); attn^2 rides the scalar engine straight from PSUM.
Compute dtype bf16 (fp32 PSUM accumulation), host pre-casts inputs.
The `rep` build parameter replicates the whole pass inside one NEFF —
used only by test.py's timing methodology.
"""

import numpy as np
import ml_dtypes

import concourse.bass as bass
import concourse.mybir as mybir
import concourse.tile as tile
from concourse import bacc
from concourse.bass_utils import run_bass_kernel_spmd

B, N, D, H = 4, 8192, 1024, 16
HD = D // H          # 64
P = 128
DC = D // P          # 8 dim chunks
NCORES = 8
R_FULL = B * N // NCORES   # 4096 rows per core
WIN = 512
EPS = 1e-5
GROUPS = [[0, 1], [2, 3], [4, 5], [6, 7]]
PIPE = 1  # phase C: out-projection trails attention by PIPE windows

bf16 = mybir.dt.bfloat16
f32 = mybir.dt.float32
AF = mybir.ActivationFunctionType
ALU = mybir.AluOpType
NPBF16 = ml_dtypes.bfloat16


def build(R=R_FULL, rep=1, sim=False, fake_coll=False):
    RT = R // P          # rowtiles
    NW = R // WIN        # windows
    RPW = WIN // P       # rowtiles per window (4)
    ndev = 1 if sim else NCORES

    nc = bacc.Bacc("TRN2", target_bir_lowering=False, debug=False,
                   enable_asserts=False, num_devices=ndev)

    xt_ext = nc.dram_tensor("xt", [DC, P, R], bf16, kind="ExternalInput").ap()
    w_ext = {n: nc.dram_tensor(n, [D, D], bf16, kind="ExternalInput").ap()
             for n in ("wk", "wv", "wq", "wu", "wo")}
    bkb_ext = nc.dram_tensor("bk_b", [P, D], f32, kind="ExternalInput").ap()
    bvb_ext = nc.dram_tensor("bv_b", [P, D], f32, kind="ExternalInput").ap()
    bob_ext = nc.dram_tensor("bo_b", [P, D], f32, kind="ExternalInput").ap()
    bqf_ext = nc.dram_tensor("bq_fm", [P, DC], f32, kind="ExternalInput").ap()
    buf_ext = nc.dram_tensor("bu_fm", [P, DC], f32, kind="ExternalInput").ap()
    lnw_ext = nc.dram_tensor("lnw_fm", [P, DC], f32, kind="ExternalInput").ap()
    lnb_ext = nc.dram_tensor("lnb_fm", [P, DC], f32, kind="ExternalInput").ap()
    out_ext = nc.dram_tensor("out", [R, D], f32, kind="ExternalOutput").ap()

    with tile.TileContext(nc, num_cores=ndev) as tc:
        with (
            tc.tile_pool(name="const", bufs=1) as cp,
            tc.tile_pool(name="wpool", bufs=3) as wp,
            tc.tile_pool(name="wps", bufs=4, space="PSUM") as wps,
            tc.tile_pool(name="accps", bufs=1, space="PSUM") as accps,
            tc.tile_pool(name="dram", bufs=1, space="DRAM") as dram,
            tc.tile_pool(name="small", bufs=2) as sp,
        ):
            # ---- constants ----
            ones128 = cp.tile([P, 1], bf16, name="ones128")
            nc.vector.memset(ones128[:], 1.0)

            for _rep in range(rep):
              # kv bounce buffers (pair AllReduce)
              kv_in = dram.tile([P, DC * P], f32, name="kv_in")
              kv_out = dram.tile([P, DC * P], f32, name="kv_out")
              kv_blk = sp.tile([P, DC * P], bf16, name="kv_blk", bufs=1)

              with tc.tile_pool(name="pcq", bufs=2) as pcq:

                  def qu_window(w, xsrc):
                      """q/u projection for one 512-token window.
                      xsrc(c) -> [P, WIN] x columns, feature chunk c."""
                      qT = pcq.tile([P, DC, WIN], bf16, name="qT",
                                    tag="qT", bufs=2)
                      uT = pcq.tile([P, DC, WIN], bf16, name="uT",
                                    tag="uT", bufs=2)
                      # all of q first: wu's DMA may still be in flight
                      for wname, bias, func, dst in (
                          ("wq", bq_fm, AF.Relu, qT),
                          ("wu", bu_fm, AF.Identity, uT),
                      ):
                          for t in range(DC):
                              ps = wps.tile([P, WIN], f32, name="pqu",
                                            tag="work")
                              for c in range(DC):
                                  nc.tensor.matmul(
                                      ps[:],
                                      w_sb[wname][:, c, t * P:(t + 1) * P],
                                      xsrc(c),
                                      start=(c == 0), stop=(c == DC - 1))
                              nc.scalar.activation(dst[:, t, :], ps[:], func,
                                                   bias=bias[:, t:t + 1],
                                                   scale=1.0)
                      return qT, uT

                  with (
                      tc.tile_pool(name="xtp", bufs=1) as xtp,
                      tc.tile_pool(name="ab", bufs=2) as ab,
                  ):
                      # first half of wk/wv unblocks the first rowtile sweep
                      # critical path to the first rowtile: 2 weight-half
                      # descriptors + window 0's 8 xT pieces (descriptor
                      # issue is serial, so keep this prefix short)
                      w_sb = {}
                      for n in ("wk", "wv"):
                          t = wp.tile([P, DC, D], bf16, name=f"{n}_sb",
                                      tag="W")
                          nc.sync.dma_start(
                              t[:, :, 0:WIN],
                              w_ext[n].rearrange("(c p) n -> p c n",
                                                 p=P)[:, :, 0:WIN])
                          w_sb[n] = t

                      xT = [xtp.tile([P, R], bf16, name=f"xT{c}", tag=f"xT{c}")
                            for c in range(DC)]
                      for c in range(DC):
                          nc.sync.dma_start(xT[c][:, 0:WIN],
                                            xt_ext[c][:, 0:WIN])

                      # k/v bias rows broadcast to all partitions
                      bk_b = cp.tile([P, D], f32, name="bk_b")
                      nc.sync.dma_start(bk_b[:], bkb_ext)
                      bv_b = cp.tile([P, D], f32, name="bv_b")
                      nc.sync.dma_start(bv_b[:], bvb_ext)

                      # rest of xT in consumption (column-window) order
                      for w in range(1, NW):
                          for c in range(DC):
                              nc.sync.dma_start(
                                  xT[c][:, w * WIN:(w + 1) * WIN],
                                  xt_ext[c][:, w * WIN:(w + 1) * WIN])

                      # second half of wk/wv, then wq + phase-C params
                      for n in ("wk", "wv"):
                          nc.sync.dma_start(
                              w_sb[n][:, :, WIN:D],
                              w_ext[n].rearrange("(c p) n -> p c n",
                                                 p=P)[:, :, WIN:D])

                      t = wp.tile([P, DC, D], bf16, name="wq_sb", tag="W")
                      nc.sync.dma_start(
                          t[:], w_ext["wq"].rearrange("(c p) n -> p c n", p=P))
                      w_sb["wq"] = t

                      bo_b = cp.tile([P, D], f32, name="bo_b")
                      nc.sync.dma_start(bo_b[:], bob_ext)
                      bq_fm = cp.tile([P, DC], f32, name="bq_fm")
                      nc.sync.dma_start(bq_fm[:], bqf_ext)
                      bu_fm = cp.tile([P, DC], f32, name="bu_fm")
                      nc.sync.dma_start(bu_fm[:], buf_ext)
                      lnw_fm = cp.tile([P, DC], f32, name="lnw_fm")
                      nc.sync.dma_start(lnw_fm[:], lnw_ext)
                      lnb_fm = cp.tile([P, DC], f32, name="lnb_fm")
                      nc.sync.dma_start(lnb_fm[:], lnb_ext)

                      # ---- phase A: k, v projections + partial kv ----
                      # Two 512-wide feature halves; kv head-pair block g
                      # lives entirely in half g//4, so each half's kv
                      # accumulation is a self-contained PSUM bank group.
                      kv_ps = accps.tile([P, DC * P], f32, name="kv_ps")
                      for half in range(2):
                          hs = slice(half * WIN, half * WIN + WIN)
                          for rt in range(RT):
                              pk = wps.tile([P, WIN], f32, name="pk",
                                            tag="work")
                              pv = wps.tile([P, WIN], f32, name="pv",
                                            tag="work")
                              for c in range(DC):
                                  st, sto = c == 0, c == DC - 1
                                  lhs = xT[c][:, rt * P:(rt + 1) * P]
                                  nc.tensor.matmul(pk[:], lhs,
                                                   w_sb["wk"][:, c, hs],
                                                   start=st, stop=sto)
                                  nc.tensor.matmul(pv[:], lhs,
                                                   w_sb["wv"][:, c, hs],
                                                   start=st, stop=sto)
                              k_bf = ab.tile([P, WIN], bf16, name="k_bf",
                                             tag="kvt", bufs=4)
                              v_bf = ab.tile([P, WIN], bf16, name="v_bf",
                                             tag="kvt", bufs=4)
                              kf = ab.tile([P, WIN], f32, name="kf", tag="kf",
                                           bufs=3)
                              # k = relu(psum + bias): DVE add, then max0+cast
                              nc.vector.tensor_tensor(kf[:], pk[:],
                                                      bk_b[:, hs], ALU.add)
                              nc.vector.tensor_scalar(k_bf[:], kf[:], 0.0,
                                                      None, op0=ALU.max)
                              nc.vector.tensor_tensor(v_bf[:], pv[:],
                                                      bv_b[:, hs], ALU.add)
                              # partial kv per head-pair: [128,128] blocks
                              for g4 in range(4):
                                  g = half * 4 + g4
                                  nc.tensor.matmul(
                                      kv_ps[:, g * P:(g + 1) * P],
                                      k_bf[:, g4 * P:(g4 + 1) * P],
                                      v_bf[:, g4 * P:(g4 + 1) * P],
                                      start=(rt == 0 and g4 == 0),
                                      stop=(rt == RT - 1 and g4 == 3),
                                  )

                          # this half's kv partial -> DRAM (collective
                          # launch overhead is ~60us, so a single AllReduce
                          # at the end beats one per half)
                          kv_sb = sp.tile([P, WIN], f32, name="kv_sb", bufs=2)
                          nc.vector.tensor_copy(kv_sb[:], kv_ps[:, hs])
                          nc.sync.dma_start(kv_in[:, hs], kv_sb[:])

                      if sim or fake_coll:
                          nc.sync.dma_start(kv_out[:], kv_in[:])
                      else:
                          nc.gpsimd.collective_compute(
                              "AllReduce", ALU.add, replica_groups=GROUPS,
                              ins=[kv_in[:]], outs=[kv_out[:]],
                          )

                      # wu/wo loads resolve their buffer-slot waits as phase A
                      # drains; emitted after the A loop so they can't block
                      # earlier DMA queue entries.
                      for n in ("wu", "wo"):
                          t = wp.tile([P, DC, D], bf16, name=f"{n}_sb",
                                      tag="W")
                          nc.sync.dma_start(
                              t[:],
                              w_ext[n].rearrange("(c p) n -> p c n", p=P))
                          w_sb[n] = t

                      # windows 0/1's q/u run during the collective, reading
                      # the still-resident phase-A xT
                      qu_tiles = {}
                      for w in range(min(2, NW)):
                          cols = slice(w * WIN, (w + 1) * WIN)
                          qu_tiles[w] = qu_window(
                              w, lambda c, cols=cols: xT[c][:, cols])

                  # ---- phase C (xT/ab space recycled from here on) ----
                  pc_cm = tc.tile_pool(name="pc", bufs=2)
                  pc = pc_cm.__enter__()
                  # x column-window ring for windows >= 2, re-streamed from
                  # DRAM; prefetched ahead of the collective-gated readback
                  # so these DMAs never queue behind it.
                  xw_tiles = {}

                  def xw_fetch(w):
                      if not 2 <= w < NW:
                          return
                      xw = pc.tile([P, DC, WIN], bf16, name="xw", tag="xw",
                                   bufs=2)
                      for c in range(DC):
                          nc.sync.dma_start(
                              xw[:, c, :],
                              xt_ext[c][:, w * WIN:(w + 1) * WIN])
                      xw_tiles[w] = xw

                  xw_fetch(2)
                  xw_fetch(3)

                  # kv: per-half readback, clamp, block-diagonal build
                  nc.vector.memset(kv_blk[:], 0.0)
                  for half in range(2):
                      kv_rb = sp.tile([P, WIN], f32, name="kv_rb", bufs=2)
                      nc.sync.dma_start(kv_rb[:],
                                        kv_out[:, half * WIN:half * WIN + WIN])
                      # clamp to [-100, 100], then |.| >= 0.01 keeping sign
                      nc.vector.tensor_scalar(kv_rb[:], kv_rb[:], -100.0,
                                              100.0, op0=ALU.max, op1=ALU.min)
                      kv_sgn = sp.tile([P, WIN], bf16, name="kv_sgn", bufs=2)
                      nc.scalar.activation(kv_sgn[:], kv_rb[:], AF.Sign)
                      nc.scalar.activation(kv_rb[:], kv_rb[:], AF.Abs)
                      nc.vector.tensor_scalar(kv_rb[:], kv_rb[:], 0.01, None,
                                              op0=ALU.max)
                      kv_cl = sp.tile([P, WIN], bf16, name="kv_cl", bufs=2)
                      nc.vector.tensor_tensor(kv_cl[:], kv_sgn[:], kv_rb[:],
                                              ALU.mult)
                      # kv_blk[0:64, g*128:g*128+64]       = kv(head 2g)
                      # kv_blk[64:128, g*128+64:g*128+128] = kv(head 2g+1)
                      for g4 in range(4):
                          g = half * 4 + g4
                          nc.vector.tensor_copy(
                              kv_blk[0:HD, g * P:g * P + HD],
                              kv_cl[0:HD, g4 * P:g4 * P + HD])
                          nc.vector.tensor_copy(
                              kv_blk[HD:P, g * P + HD:(g + 1) * P],
                              kv_cl[HD:P, g4 * P + HD:(g4 + 1) * P])

                  zw_tiles = {}
                  for w in range(NW + PIPE):
                    if w < NW:
                      qT_w, uT_w = qu_tiles.pop(w)

                      # attention + attn^2 (ACT, straight from PSUM)
                      attn = pc.tile([P, DC, WIN], bf16, name="attn",
                                     tag="attn")
                      at2 = pc.tile([P, DC, WIN], bf16, name="at2",
                                    tag="at2", bufs=1)
                      for g in range(DC):
                          aps = wps.tile([P, WIN], f32, name="aps", tag="work")
                          nc.tensor.matmul(aps[:],
                                           kv_blk[:, g * P:(g + 1) * P],
                                           qT_w[:, g, :], start=True,
                                           stop=True)
                          nc.scalar.activation(attn[:, g, :], aps[:], AF.Copy)
                          nc.vector.tensor_tensor(at2[:, g, :], attn[:, g, :],
                                                  attn[:, g, :], ALU.mult)
                      # LN stats: per-column sums over all 1024 dims
                      s_ps = wps.tile([1, WIN], f32, name="s_ps", tag="stat",
                                      bufs=2)
                      q_ps = wps.tile([1, WIN], f32, name="q_ps", tag="stat",
                                      bufs=2)
                      for g in range(DC):
                          nc.tensor.matmul(s_ps[:], ones128[:], attn[:, g, :],
                                           start=(g == 0), stop=(g == DC - 1))
                          nc.tensor.matmul(q_ps[:], ones128[:], at2[:, g, :],
                                           start=(g == 0), stop=(g == DC - 1))

                      # next window's q/u + x prefetch (emitted between the
                      # stats matmuls and the out-projection so the PE always
                      # has independent work while the LN chain runs)
                      if w + 2 < NW:
                          xw = xw_tiles.pop(w + 2)
                          qu_tiles[w + 2] = qu_window(
                              w + 2, lambda c, xw=xw: xw[:, c, :])
                      xw_fetch(w + 4)

                      mean_t = pc.tile([1, WIN], f32, name="mean_t",
                                       tag="mean_t")
                      var_t = pc.tile([1, WIN], f32, name="var_t", tag="var_t")
                      nc.vector.tensor_scalar(mean_t[:], s_ps[:], 1.0 / D,
                                              None, op0=ALU.mult)   # mean
                      # var + eps = E[x^2] - mean^2 + eps
                      nc.vector.tensor_tensor(var_t[:], mean_t[:], mean_t[:],
                                              ALU.mult)             # mean^2
                      nc.vector.scalar_tensor_tensor(var_t[:], q_ps[:],
                                                     1.0 / D, var_t[:],
                                                     ALU.mult, ALU.subtract)
                      nc.vector.tensor_scalar(var_t[:], var_t[:], EPS, None,
                                              op0=ALU.add)
                      nc.vector.reciprocal(var_t[:], var_t[:])
                      rstd = pc.tile([1, WIN], bf16, name="rstd", tag="rstd")
                      nc.scalar.activation(rstd[:], var_t[:], AF.Sqrt)
                      shp = pc.tile([1, WIN], bf16, name="shp", tag="shp")
                      # shiftpre = -mean * rstd
                      nc.vector.scalar_tensor_tensor(shp[:], mean_t[:], -1.0,
                                                     rstd[:], ALU.mult,
                                                     ALU.mult)
                      # broadcast per-column stats to all partitions (GPSIMD)
                      rstd_b = pc.tile([P, WIN], bf16, name="rstd_b",
                                       tag="rstd_b")
                      nc.gpsimd.partition_broadcast(rstd_b[:], rstd[:])
                      shp_b = pc.tile([P, WIN], bf16, name="shp_b",
                                      tag="shp_b")
                      nc.gpsimd.partition_broadcast(shp_b[:], shp[:])

                      # z = ((attn * rstd + shiftpre) * lnw + lnb) * u
                      # (the per-partition lnw/lnb affine rides ACT)
                      def z_emit(zdst, g, cols):
                          zt = pc.tile([P, WIN], bf16, name="zt", tag="zt",
                                       bufs=3)
                          n = cols.stop - cols.start
                          nc.vector.tensor_tensor(zt[:, 0:n],
                                                  attn[:, g, cols],
                                                  rstd_b[:, cols], ALU.mult)
                          nc.vector.tensor_tensor(zt[:, 0:n], zt[:, 0:n],
                                                  shp_b[:, cols], ALU.add)
                          zt2 = pc.tile([P, WIN], bf16, name="zt2", tag="zt2",
                                        bufs=3)
                          nc.scalar.activation(zt2[:, 0:n], zt[:, 0:n],
                                               AF.Identity,
                                               bias=lnb_fm[:, g:g + 1],
                                               scale=lnw_fm[:, g:g + 1])
                          nc.vector.tensor_tensor(zdst, zt2[:, 0:n],
                                                  uT_w[:, g, cols], ALU.mult)

                      if w < NW - 1:
                          zw = pc.tile([P, DC, WIN], bf16, name="zw",
                                       tag="zw", bufs=PIPE + 1)
                          for g in range(DC):
                              z_emit(zw[:, g, :], g, slice(0, WIN))
                          zw_tiles[w] = zw

                    if w >= PIPE and w - PIPE == NW - 1:
                          # fused tail: z per column-quarter, immediately
                          # followed by that quarter's out-projection, so the
                          # final Wo pipelines with the final z instead of
                          # serializing after it
                          for j in range(RPW):
                              zwq = pc.tile([P, DC, P], bf16, name="zwq",
                                            tag="zwq", bufs=2)
                              cq = slice(j * P, (j + 1) * P)
                              for g in range(DC):
                                  z_emit(zwq[:, g, :], g, cq)
                              o0 = wps.tile([P, WIN], f32, name="o0",
                                            tag="work")
                              o1 = wps.tile([P, WIN], f32, name="o1",
                                            tag="work")
                              for c in range(DC):
                                  nc.tensor.matmul(o0[:], zwq[:, c, :],
                                                   w_sb["wo"][:, c, 0:WIN],
                                                   start=(c == 0),
                                                   stop=(c == DC - 1))
                                  nc.tensor.matmul(o1[:], zwq[:, c, :],
                                                   w_sb["wo"][:, c, WIN:D],
                                                   start=(c == 0),
                                                   stop=(c == DC - 1))
                              osb = pc.tile([P, D], f32, name="osb",
                                            tag="osb", bufs=2)
                              nc.vector.scalar_tensor_tensor(
                                  osb[:, 0:WIN], o0[:], 1.0, bo_b[:, 0:WIN],
                                  ALU.mult, ALU.add)
                              nc.vector.scalar_tensor_tensor(
                                  osb[:, WIN:D], o1[:], 1.0, bo_b[:, WIN:D],
                                  ALU.mult, ALU.add)
                              rt = (NW - 1) * RPW + j
                              nc.sync.dma_start(
                                  out_ext[rt * P:(rt + 1) * P, :], osb[:])

                    if w >= PIPE and w - PIPE < NW - 1:
                      # out = z @ Wo + bo for window w-PIPE (row-major out,
                      # zT chunks stationary)
                      wc = w - PIPE
                      zw = zw_tiles.pop(wc)
                      for j in range(RPW):
                          o0 = wps.tile([P, WIN], f32, name="o0", tag="work")
                          o1 = wps.tile([P, WIN], f32, name="o1", tag="work")
                          for c in range(DC):
                              lhs = zw[:, c, j * P:(j + 1) * P]
                              nc.tensor.matmul(o0[:], lhs,
                                               w_sb["wo"][:, c, 0:WIN],
                                               start=(c == 0),
                                               stop=(c == DC - 1))
                              nc.tensor.matmul(o1[:], lhs,
                                               w_sb["wo"][:, c, WIN:D],
                                               start=(c == 0),
                                               stop=(c == DC - 1))
                          osb = pc.tile([P, D], f32, name="osb", tag="osb",
                                        bufs=2)
                          nc.vector.scalar_tensor_tensor(osb[:, 0:WIN], o0[:],
                                                         1.0, bo_b[:, 0:WIN],
                                                         ALU.mult, ALU.add)
                          nc.vector.scalar_tensor_tensor(osb[:, WIN:D], o1[:],
                                                         1.0, bo_b[:, WIN:D],
                                                         ALU.mult, ALU.add)
                          rt = wc * RPW + j
                          nc.sync.dma_start(out_ext[rt * P:(rt + 1) * P, :],
                                            osb[:])

                  pc_cm.__exit__(None, None, None)

    nc.compile()
    return nc


def make_in_maps(query, Wq, bq, Wk, bk, Wv, bv, Wu, bu, Wo, bo, ln_w, ln_b,
                 R=R_FULL):
    xs = query.reshape(-1, D).astype(NPBF16)
    common = {
        "wk": np.ascontiguousarray(Wk).astype(NPBF16),
        "wv": np.ascontiguousarray(Wv).astype(NPBF16),
        "wq": np.ascontiguousarray(Wq).astype(NPBF16),
        "wu": np.ascontiguousarray(Wu).astype(NPBF16),
        "wo": np.ascontiguousarray(Wo).astype(NPBF16),
        "bk_b": np.ascontiguousarray(
            np.broadcast_to(bk.astype(np.float32), (P, D))),
        "bv_b": np.ascontiguousarray(
            np.broadcast_to(bv.astype(np.float32), (P, D))),
        "bo_b": np.ascontiguousarray(
            np.broadcast_to(bo.astype(np.float32), (P, D))),
        "bq_fm": np.ascontiguousarray(bq.astype(np.float32).reshape(DC, P).T),
        "bu_fm": np.ascontiguousarray(bu.astype(np.float32).reshape(DC, P).T),
        "lnw_fm": np.ascontiguousarray(ln_w.astype(np.float32).reshape(DC, P).T),
        "lnb_fm": np.ascontiguousarray(ln_b.astype(np.float32).reshape(DC, P).T),
    }
    return [dict(common, xt=np.ascontiguousarray(
                xs[c * R:(c + 1) * R].T.reshape(DC, P, R)))
            for c in range(NCORES)]


_NC_CACHE = {}


def kernel(query, Wq, bq, Wk, bk, Wv, bv, Wu, bu, Wo, bo, ln_w, ln_b):
    query = np.asarray(query, dtype=np.float32)
    if "nc" not in _NC_CACHE:
        _NC_CACHE["nc"] = build()
    nc = _NC_CACHE["nc"]
    in_maps = make_in_maps(query, np.asarray(Wq), np.asarray(bq),
                           np.asarray(Wk), np.asarray(bk),
                           np.asarray(Wv), np.asarray(bv),
                           np.asarray(Wu), np.asarray(bu),
                           np.asarray(Wo), np.asarray(bo),
                           np.asarray(ln_w), np.asarray(ln_b))
    res = run_bass_kernel_spmd(nc, in_maps, list(range(NCORES)))
    out = np.empty((B * N, D), np.float32)
    for c in range(NCORES):
        out[c * R_FULL:(c + 1) * R_FULL] = res.results[c]["out"]
    return out.reshape(B, N, D)
